# revision 1
# baseline (speedup 1.0000x reference)
"""Trainium2 Bass kernel for nn_EnhancedQuantumLayer (10-qubit, 4-layer
variational circuit, batch 512, Z-expectations output).

Strategy (data parallel over 8 cores, 64 samples/core):
  - Feature map is a product state: per (sample, qubit) 2-vector v computed
    by a 10-step RZ/RX recursion on [64, 40] tiles (DVE) with sin/cos planes
    from the ACT engine.
  - Statevector [64, 1024] (complex -> separate re/im fp32 planes) is held as
    [128, 512] tiles: partition = (cA=b//16, h), free = (u=b%16, l), where
    h = 5 "A" qubits (order q4,q0,q1,q2,q3 MSB->LSB) and l = 5 "L" qubits
    (q5..q9).  A 32x32 block transpose (DVE StreamTranspose) flips between
    layout A (h on partitions) and layout B (l on partitions).
  - Each layer applies block-diag(4x 32x32) stationaries on the TensorEngine
    in fp32r (full-rate fp32): A-side gates in layout A, L-side gates in
    layout B with two q4-conditional stationaries (this absorbs the
    entangling CZCNOT(4,5) gate); all other CZCNOT gates are signed
    permutations folded into the stationaries / measurement weights on host.
  - Measurement: |amp|^2 then two small sign-weight matmuls + tiny DMAs.

Host precompute is theta-only (36 128x128 stationaries) -- O(1) in batch.
"""

import math

import numpy as np

N_QUBITS = 10
N_LAYERS = 4
FREQS = (1.0, 2.0, 4.0, 8.0, 16.0)
PI = float(np.pi)
B_TOTAL = 512
B_CORE = 64
N_CORES = 8

H_QUBITS = [4, 0, 1, 2, 3]   # kron order (MSB first) for h index
L_QUBITS = [5, 6, 7, 8, 9]

CZCNOT = np.array([[1, 0, 0, 0],
                   [0, 1, 0, 0],
                   [0, 0, 0, -1],
                   [0, 0, 1, 0]], dtype=np.complex128)
G5 = np.array([[0, -1], [1, 0]], dtype=np.complex128)


# ---------------------------------------------------------------- host math
def _rz(phi):
    return np.array([[np.exp(-0.5j * phi), 0], [0, np.exp(0.5j * phi)]],
                    dtype=np.complex128)


def _rx(th):
    c, s = np.cos(th / 2), np.sin(th / 2)
    return np.array([[c, -1j * s], [-1j * s, c]], dtype=np.complex128)


def _ry(th):
    c, s = np.cos(th / 2), np.sin(th / 2)
    return np.array([[c, -s], [s, c]], dtype=np.complex128)


def _kron_list(ms):
    out = ms[0]
    for m in ms[1:]:
        out = np.kron(out, m)
    return out


def _embed_2q(space_qubits, qa, qb, M4):
    n = len(space_qubits)
    dim = 2 ** n
    pa, pb = space_qubits.index(qa), space_qubits.index(qb)
    out = np.zeros((dim, dim), dtype=np.complex128)
    for idx in range(dim):
        bits = [(idx >> (n - 1 - i)) & 1 for i in range(n)]
        col4 = 2 * bits[pa] + bits[pb]
        for row4 in range(4):
            val = M4[row4, col4]
            if val != 0:
                nb = bits.copy()
                nb[pa], nb[pb] = row4 >> 1, row4 & 1
                ridx = sum(bit << (n - 1 - i) for i, bit in enumerate(nb))
                out[ridx, idx] += val
    return out


def _embed_1q(space_qubits, q, M2):
    return _kron_list([M2 if sq == q else np.eye(2) for sq in space_qubits])


A6 = [4, "b4", 0, 1, 2, 3]
L6 = [4, 5, 6, 7, 8, 9]
_E_evenA6 = _embed_2q(A6, 0, 1, CZCNOT) @ _embed_2q(A6, 2, 3, CZCNOT)
_E_oddA6 = _embed_2q(A6, 3, 4, CZCNOT) @ _embed_2q(A6, 1, 2, CZCNOT)
_PermA6 = _E_oddA6 @ _E_evenA6
_E_evenL6 = _embed_2q(L6, 6, 7, CZCNOT) @ _embed_2q(L6, 8, 9, CZCNOT)
_E_oddL6 = _embed_2q(L6, 7, 8, CZCNOT) @ _embed_2q(L6, 5, 6, CZCNOT)
_CG64 = _embed_2q(L6, 4, 5, CZCNOT)


def _layer_matrices6(theta):
    ang = np.tanh(theta.astype(np.float64)) * PI
    S_A, S_L = [], []
    for layer in range(N_LAYERS):
        U = []
        for q in range(10):
            a0, a1, a2 = ang[layer, q]
            U.append(_rx(a0 * 0.5) @ _rz(a2) @ _ry(a1) @ _rz(a0))
        UA6 = _kron_list([U[4], np.eye(2), U[0], U[1], U[2], U[3]])
        UL6 = _kron_list([np.eye(2), U[5], U[6], U[7], U[8], U[9]])
        S_A.append(UA6 if layer == 0 else UA6 @ _PermA6)
        S_L.append(_E_oddL6 @ _E_evenL6 @ _CG64 @ UL6)
    return S_A, S_L


def _measurement_weights6():
    W1 = np.zeros((128, 32), dtype=np.float32)
    for b5 in range(2):
        for q4 in range(2):
            for l in range(32):
                p = 64 * b5 + 32 * q4 + l
                W1[p, 16 * b5 + 0] = 1.0 - 2.0 * q4
                for j in range(5):
                    W1[p, 16 * b5 + 1 + j] = 1.0 - 2.0 * ((l >> (4 - j)) & 1)
                W1[p, 16 * b5 + 6] = 1.0
    W2 = np.zeros((32, 16), dtype=np.float32)
    for b4 in range(2):
        for h4 in range(16):
            p = 16 * b4 + h4
            s = [1.0 - 2.0 * ((h4 >> (3 - i)) & 1) for i in range(4)]
            W2[p, 8 * b4 + 0] = s[0]
            W2[p, 8 * b4 + 1] = s[1] * s[0]
            W2[p, 8 * b4 + 2] = s[2] * s[1] * s[0]
            W2[p, 8 * b4 + 3] = s[3] * s[2]
            W2[p, 8 * b4 + 4] = 1.0
    return W1, W2


def _host_weights(theta):
    """wstack [24, 128, 128] fp32: per layer [Ar, Ain, Ai, Lr, Lin, Li],
    each kron(I2, S6).{comp}.T"""
    S_A, S_L = _layer_matrices6(theta)
    I2 = np.eye(2)
    mats = []
    for layer in range(N_LAYERS):
        for S in [S_A[layer], S_L[layer]]:
            full = np.kron(I2, S)
            mats.append(full.real.T)
            mats.append((-full.imag).T)
            mats.append(full.imag.T)
    return np.ascontiguousarray(np.stack(mats).astype(np.float32))


# ------------------------------------------------------------- bass builder
_BUILD_CACHE = {}


def _build_module():
    """Build the (input-independent) Bass module. Returns (nc, names)."""
    import concourse.bass as bass
    import concourse.mybir as mybir
    from concourse import bacc
    from concourse.tile import TileContext

    f32 = mybir.dt.float32
    f32r = mybir.dt.float32r
    AF = mybir.ActivationFunctionType
    OP = mybir.AluOpType

    nc = bacc.Bacc("TRN2", target_bir_lowering=False, debug=False)

    xin = nc.dram_tensor("xin", [B_CORE, 10], f32, kind="ExternalInput").ap()
    wstack = nc.dram_tensor("wstack", [24, 128, 128], f32,
                            kind="ExternalInput").ap()
    out_d = nc.dram_tensor("out", [B_CORE, 10], f32, kind="ExternalOutput").ap()

    # ---- inline constants
    v0_np = np.zeros((64, 40), dtype=np.float32)
    v0_np[:, 0::4] = 1.0  # alpha_re = 1
    # trig-arg planes: rows 0-5 sin(c_j x), 6-11 cos(c_j x), c_j = 0.25*2^j
    # arg = (c*x + b + 5pi) mod 2pi, then ACT Sin with bias -pi
    cs_mult = np.zeros((12, 10), dtype=np.float32)
    cs_bias = np.zeros((12, 10), dtype=np.float32)
    for j in range(6):
        cs_mult[j] = 0.25 * 2 ** j
        cs_mult[6 + j] = 0.25 * 2 ** j
        cs_bias[j] = 0.0
        cs_bias[6 + j] = 0.5 * PI
    mult_np = np.tile(cs_mult.reshape(1, 120), (64, 1)).astype(np.float32)
    bias_np = np.tile(cs_bias.reshape(1, 120), (64, 1)).astype(np.float32)
    W1_np, W2_np = _measurement_weights6()
    # replication matrix: row r=(b5,b4) -> partitions with those sample bits
    E4_np = np.zeros((4, 128), dtype=np.float32)
    for b5 in range(2):
        for b4 in range(2):
            for q4 in range(2):
                p0 = 64 * b5 + 32 * q4 + 16 * b4
                E4_np[2 * b5 + b4, p0:p0 + 16] = 1.0
    # pack all small consts into one [128, 456] tensor -> single DMA
    cpack = np.zeros((128, 456), dtype=np.float32)
    cpack[:, 0:32] = W1_np
    cpack[0:32, 32:48] = W2_np
    cpack[0:4, 48:176] = E4_np
    cpack[0:64, 176:216] = v0_np
    cpack[0:64, 216:336] = mult_np
    cpack[0:64, 336:456] = bias_np
    cpack_c = nc.inline_tensor(cpack, name="cpack").ap()


    scr = nc.dram_tensor("scr", [2, 64, 64], f32)  # Internal DRAM bounce

    with TileContext(nc) as tc:
        with (
            tc.tile_pool(name="wpool", bufs=1) as wpool,
            tc.tile_pool(name="sb", bufs=2) as sb,
            tc.tile_pool(name="small", bufs=2) as sm,
            tc.tile_pool(name="psA", bufs=2, space="PSUM") as psA,
            tc.tile_pool(name="psB", bufs=1, space="PSUM") as psB,
        ):
            # ---- weights + consts into SBUF (2 DMA dispatches total)
            wt = wpool.tile([128, 24 * 128], f32, tag="w")
            nc.sync.dma_start(
                wt[:].rearrange("p (m c) -> p m c", c=128).bitcast(f32r),
                wstack.transpose([1, 0, 2]).bitcast(f32r))

            def W(m):
                return wt[:, 128 * m:128 * m + 128].bitcast(f32r)

            ct = wpool.tile([128, 456], f32, tag="cp")
            nc.scalar.dma_start(ct[:], cpack_c)
            w1_t = ct[:, 0:32]
            w2_t = ct[0:32, 32:48]
            e4_t = ct[0:4, 48:176]
            v0_v = ct[0:64, 176:216]
            mult_v = ct[0:64, 216:336]
            bias_v = ct[0:64, 336:456]

            # ---- feature map: x = tanh(xin); trig table; v-recursion
            sx = sm.tile([64, 10], f32, tag="sx")
            nc.sync.dma_start(sx[:], xin)
            xt = sm.tile([64, 10], f32, tag="xt")
            nc.scalar.activation(xt[:], sx[:], AF.Tanh)



            # trig table tb: rows 0-5 = sin(c_j x), 6-11 = -sin, 12-17 = cos
            # (row r occupies cols 10r..10r+10)
            xb12 = (xt[:].unsqueeze(1).broadcast_to((64, 12, 10)))
            ma = sm.tile([64, 120], f32, tag="ma")
            nc.vector.tensor_tensor(
                ma[:].rearrange("p (r q) -> p r q", q=10), xb12, mult_v
                .rearrange("p (r q) -> p r q", q=10), OP.mult)
            nc.vector.tensor_tensor(ma[:], ma[:], bias_v, OP.add)
            # range reduce to [-pi, pi]: k = round(ma/2pi) via magic-number
            MAGIC = 1.5 * 2 ** 23
            kk = sm.tile([64, 120], f32, tag="kk")
            nc.vector.tensor_scalar(kk[:], ma[:], 1.0 / (2.0 * PI), MAGIC,
                                    OP.mult, OP.add)
            nc.vector.tensor_scalar(kk[:], kk[:], MAGIC, None, OP.subtract)
            nc.vector.scalar_tensor_tensor(ma[:], kk[:], -2.0 * PI, ma[:],
                                           OP.mult, OP.add)
            PCLAMP = PI * (1.0 - 1e-6)
            nc.vector.tensor_scalar(ma[:], ma[:], PCLAMP, -PCLAMP,
                                    OP.min, OP.max)
            tb = sm.tile([64, 180], f32, tag="tb180")
            # sin & cos rows: Sin(ma - pi); layout: [s(0:60) | cos(60:120)]
            # goes to tb rows 0-5 (s) and 12-17 (cos)
            nc.scalar.activation(tb[:, 0:60], ma[:, 0:60], AF.Sin)
            nc.scalar.activation(tb[:, 120:180], ma[:, 60:120], AF.Sin)
            # ns rows 6-11 = -s
            nc.scalar.activation(tb[:, 60:120], tb[:, 0:60], AF.Copy,
                                 scale=-1.0)


            tb_v = tb[:].rearrange("p (r q) -> p r q", q=10)  # [64, 18, 10]
            v_cur = None
            for k in range(10):
                is_rz = (k % 2 == 0)
                lv = k // 2 + 1 if is_rz else k // 2
                # cos plane: direct bcast view of cos row (12+lv)
                cplane = (tb_v[:, 12 + lv, :].unsqueeze(2)
                          .broadcast_to((64, 10, 4))
                          .rearrange("p q (a b) -> p q a b", a=2))
                t1 = sm.tile([64, 40], f32, tag="t1")
                t2 = sm.tile([64, 40], f32, tag="t2")
                t1v = t1[:].rearrange("p (q a b) -> p q a b", a=2, b=2)
                t2v = t2[:].rearrange("p (q a b) -> p q a b", a=2, b=2)
                vsrc = v0_v if v_cur is None else v_cur[:]
                vv = vsrc.rearrange("p (q a b) -> p q a b", a=2, b=2)
                nc.vector.tensor_tensor(t1v, vv, cplane, OP.mult)
                if is_rz:
                    # t2 comps: (+s*ai, -s*ar, -s*bi, +s*br):
                    # alpha-half in1 rows (s, ns); beta-half rows (ns, s)
                    vpart = vv[:, :, :, ::-1]
                    s_alpha = (tb_v[:, lv:lv + 7:6, :].transpose([0, 2, 1])
                               .unsqueeze(2))       # [64, 10, 1, 2] (s, ns)
                    s_beta = (tb_v[:, lv + 6:lv - 1:-6, :]
                              .transpose([0, 2, 1]).unsqueeze(2))
                    nc.gpsimd.tensor_tensor(t2v[:, :, 0:1, :],
                                            vpart[:, :, 0:1, :], s_alpha,
                                            OP.mult)
                    nc.gpsimd.tensor_tensor(t2v[:, :, 1:2, :],
                                            vpart[:, :, 1:2, :], s_beta,
                                            OP.mult)
                else:
                    # t2 comps: (+s*bi, -s*br, +s*ai, -s*ar): b-stride (s, ns)
                    vpart = vv[:, :, ::-1, ::-1]
                    sview = (tb_v[:, lv:lv + 7:6, :].transpose([0, 2, 1])
                             .unsqueeze(2).broadcast_to((64, 10, 2, 2)))
                    nc.gpsimd.tensor_tensor(t2v, vpart, sview, OP.mult)
                v_nxt = sm.tile([64, 40], f32, tag="vb" if k % 2 == 0 else "va")
                nc.vector.tensor_tensor(v_nxt[:], t1[:], t2[:], OP.add)
                v_cur = v_nxt

            # ---- H/L doubling: G tiles [64, 64], H cols 0:32, L cols 32:64
            g_r = sm.tile([64, 64], f32, tag="gra")
            g_i = sm.tile([64, 64], f32, tag="gia")
            # step 0: copy v[q4] into H slot, v[q5] into L slot
            # v comps for q: (ar, ai, br, bi) at 4q+0..3; vr[q,t] = 4q+2t,
            # vi[q,t] = 4q+2t+1
            vvq = v_cur[:].rearrange("p (q t c) -> p q t c", t=2, c=2)
            g_r0 = g_r[:].rearrange("p (s x) -> p s x", s=2)[:, :, 0:2]
            g_i0 = g_i[:].rearrange("p (s x) -> p s x", s=2)[:, :, 0:2]
            nc.vector.tensor_copy(g_r0, vvq[:, 4:6, :, 0])
            nc.vector.tensor_copy(g_i0, vvq[:, 4:6, :, 1])
            for j in range(1, 5):
                w = 2 ** j
                qH, qL = H_QUBITS[j], L_QUBITS[j]  # qL - qH == 6
                ptA = sm.tile([64, 8 * w], f32, tag="ptA")
                ptB = sm.tile([64, 8 * w], f32, tag="ptB")
                gr_b = (g_r[:].rearrange("p (s x) -> p s x", s=2)[:, :, 0:w]
                        .unsqueeze(3).broadcast_to((64, 2, w, 2)))
                gi_b = (g_i[:].rearrange("p (s x) -> p s x", s=2)[:, :, 0:w]
                        .unsqueeze(3).broadcast_to((64, 2, w, 2)))
                vsel = vvq[:, qH:qH + 7:6]          # [64, 2q, 2t, 2c]
                vA = (vsel.transpose([0, 3, 1, 2])  # [64, c(r,i), q, t]
                      .unsqueeze(3).broadcast_to((64, 2, 2, w, 2)))
                vB = (vsel[:, :, :, ::-1].transpose([0, 3, 1, 2])
                      .unsqueeze(3).broadcast_to((64, 2, 2, w, 2)))
                ptA_v = ptA[:].rearrange("p (c s x t) -> p c s x t",
                                         c=2, s=2, t=2)
                ptB_v = ptB[:].rearrange("p (c s x t) -> p c s x t",
                                         c=2, s=2, t=2)
                for c in range(2):
                    nc.vector.tensor_tensor(ptA_v[:, c], gr_b, vA[:, c],
                                            OP.mult)
                    nc.vector.tensor_tensor(ptB_v[:, c], gi_b, vB[:, c],
                                            OP.mult)
                g2_r = sm.tile([64, 64], f32, tag="grb" if j % 2 else "gra")
                g2_i = sm.tile([64, 64], f32, tag="gib" if j % 2 else "gia")
                g2r_v = g2_r[:].rearrange("p (s h t) -> p s h t",
                                          s=2, t=2)[:, :, 0:w, :]
                g2i_v = g2_i[:].rearrange("p (s h t) -> p s h t",
                                          s=2, t=2)[:, :, 0:w, :]
                nc.vector.tensor_tensor(g2r_v, ptA_v[:, 0], ptB_v[:, 0],
                                        OP.subtract)
                nc.vector.tensor_tensor(g2i_v, ptA_v[:, 1], ptB_v[:, 1],
                                        OP.add)
                g_r, g_i = g2_r, g2_i

            # ---- bounce G through DRAM to repack
            nc.sync.dma_start(scr.ap()[0], g_r[:])
            nc.sync.dma_start(scr.ap()[1], g_i[:])

            # H' [128 = 64 b5 + 32 q4 + 16 b4 + h4, 16 = u]
            hp_r = sm.tile([128, 16], f32, tag="hpr")
            hp_i = sm.tile([128, 16], f32, tag="hpi")
            for comp, hp in ((0, hp_r), (1, hp_i)):
                for b5 in range(2):
                    for q4 in range(2):
                        for b4 in range(2):
                            p0 = 64 * b5 + 32 * q4 + 16 * b4
                            r0 = 32 * b5 + 16 * b4
                            src = (scr.ap()[comp, r0:r0 + 16,
                                            16 * q4:16 * q4 + 16]
                                   .rearrange("u h -> h u"))
                            nc.scalar.dma_start(hp[p0:p0 + 16, :], src)
            # L'' [4 = (b5, b4), 512 = (u, l)]
            lpp_r = sm.tile([4, 512], f32, tag="lppr")
            lpp_i = sm.tile([4, 512], f32, tag="lppi")
            for comp, lpp in ((0, lpp_r), (1, lpp_i)):
                src = (scr.ap()[comp, :, 32:64]
                       .rearrange("(r u) l -> r u l", u=16))
                nc.sync.dma_start(lpp[:].rearrange("c (u l) -> c u l", l=32)
                                  .bitcast(f32r), src.bitcast(f32r))

            # LRep via E4 matmul: [128, 512] psum, per comp
            lr_r = psA.tile([128, 512], f32, tag="yr")
            lr_i = psA.tile([128, 512], f32, tag="yi")
            nc.tensor.matmul(lr_r[:], e4_t.bitcast(f32r),
                             lpp_r[:].bitcast(f32r), start=True, stop=True)
            nc.tensor.matmul(lr_i[:], e4_t.bitcast(f32r),
                             lpp_i[:].bitcast(f32r), start=True, stop=True)

            # ---- X build: X = Hbcast * LRep (complex), layout A
            x_r = sb.tile([128, 512], f32, tag="xr")
            x_i = sb.tile([128, 512], f32, tag="xi")
            ta = sb.tile([128, 512], f32, tag="ta")
            tb = sb.tile([128, 512], f32, tag="tb")
            hpr_b = hp_r[:].unsqueeze(2).broadcast_to((128, 16, 32))
            hpi_b = hp_i[:].unsqueeze(2).broadcast_to((128, 16, 32))
            lrr_v = lr_r[:].rearrange("p (u l) -> p u l", l=32)
            lri_v = lr_i[:].rearrange("p (u l) -> p u l", l=32)
            ta_v = ta[:].rearrange("p (u l) -> p u l", l=32)
            tb_v = tb[:].rearrange("p (u l) -> p u l", l=32)
            xr_v = x_r[:].rearrange("p (u l) -> p u l", l=32)
            xi_v = x_i[:].rearrange("p (u l) -> p u l", l=32)
            nc.vector.tensor_tensor(ta_v, hpr_b, lrr_v, OP.mult)
            nc.vector.tensor_tensor(tb_v, hpi_b, lri_v, OP.mult)
            nc.vector.tensor_tensor(xr_v.bitcast(f32r), ta_v, tb_v,
                                    OP.subtract)
            nc.vector.tensor_tensor(ta_v, hpr_b, lri_v, OP.mult)
            nc.vector.tensor_tensor(tb_v, hpi_b, lrr_v, OP.mult)
            nc.vector.tensor_tensor(xi_v.bitcast(f32r), ta_v, tb_v, OP.add)

            # ---- layers
            zr = zi = None
            for layer in range(N_LAYERS):
                base = 6 * layer
                yr = psA.tile([128, 512], f32, tag="yr")
                yi = psA.tile([128, 512], f32, tag="yi")
                xr_r32 = x_r[:].bitcast(f32r)
                xi_r32 = x_i[:].bitcast(f32r)
                nc.tensor.matmul(yr[:], W(base + 0), xr_r32,
                                 start=True, stop=False)
                nc.tensor.matmul(yr[:], W(base + 1), xi_r32,
                                 start=False, stop=True)
                nc.tensor.matmul(yi[:], W(base + 0), xi_r32,
                                 start=True, stop=False)
                nc.tensor.matmul(yi[:], W(base + 2), xr_r32,
                                 start=False, stop=True)
                # flip to layout B (f32), then ACT rounds to f32r
                b0_r = sb.tile([128, 512], f32, tag="b0r")
                b0_i = sb.tile([128, 512], f32, tag="b0i")
                nc.vector.transpose(b0_r[:], yr[:])
                nc.vector.transpose(b0_i[:], yi[:])
                b_r = sb.tile([128, 512], f32, tag="br")
                b_i = sb.tile([128, 512], f32, tag="bi")
                nc.scalar.activation(b_r[:].bitcast(f32r), b0_r[:], AF.Copy)
                nc.gpsimd.tensor_copy(b_i[:].bitcast(f32r), b0_i[:])
                # L-side (E45 folded into the 64x64 stationary blocks)
                zr = psB.tile([128, 512], f32, tag="zr")
                zi = psB.tile([128, 512], f32, tag="zi")
                br_r32 = b_r[:].bitcast(f32r)
                bi_r32 = b_i[:].bitcast(f32r)
                nc.tensor.matmul(zr[:], W(base + 3), br_r32,
                                 start=True, stop=False)
                nc.tensor.matmul(zr[:], W(base + 4), bi_r32,
                                 start=False, stop=True)
                nc.tensor.matmul(zi[:], W(base + 3), bi_r32,
                                 start=True, stop=False)
                nc.tensor.matmul(zi[:], W(base + 5), br_r32,
                                 start=False, stop=True)
                if layer < N_LAYERS - 1:
                    x0_r = sb.tile([128, 512], f32, tag="x0r")
                    x0_i = sb.tile([128, 512], f32, tag="x0i")
                    nc.vector.transpose(x0_r[:], zr[:])
                    nc.vector.transpose(x0_i[:], zi[:])
                    x_r = sb.tile([128, 512], f32, tag="xr")
                    x_i = sb.tile([128, 512], f32, tag="xi")
                    nc.scalar.activation(x_r[:].bitcast(f32r), x0_r[:],
                                         AF.Copy)
                    nc.gpsimd.tensor_copy(x_i[:].bitcast(f32r), x0_i[:])

            # ---- measurement (layout B)
            p_r = sb.tile([128, 512], f32, tag="br")
            p_i = sb.tile([128, 512], f32, tag="bi")
            nc.scalar.square(p_r[:], zr[:])
            nc.scalar.square(p_i[:], zi[:])
            p_t = sb.tile([128, 512], f32, tag="ta")
            nc.vector.tensor_tensor(p_t[:].bitcast(f32r), p_r[:], p_i[:],
                                    OP.add)

            o1 = psA.tile([32, 512], f32, tag="yr")
            nc.tensor.matmul(o1[:], w1_t.bitcast(f32r),
                             p_t[:].bitcast(f32r), start=True, stop=True)
            o1t0 = sb.tile([32, 512], f32, tag="tb")
            nc.vector.transpose(o1t0[:], o1[:])
            o1t = sb.tile([32, 512], f32, tag="o1t")
            nc.scalar.activation(o1t[:].bitcast(f32r), o1t0[:], AF.Copy)
            o2 = psA.tile([16, 512], f32, tag="yi")
            nc.tensor.matmul(o2[:], w2_t.bitcast(f32r),
                             o1t[:].bitcast(f32r), start=True, stop=True)
            res = sm.tile([16, 512], f32, tag="res")
            nc.vector.tensor_copy(res[:], o2[:])

            # ---- gather: out[b = 32 b5 + 16 b4 + u, q]
            # O2[8 b4 + j', 32 u + 16 b5 + j]:
            #  q0..3 -> (j'=q, j=6); q4 -> (j'=3, j=0); q5..9 -> (j'=4, j=q-4)
            res_ap = res[:]
            for b5 in range(2):
                for b4 in range(2):
                    rows = out_d[32 * b5 + 16 * b4:32 * b5 + 16 * b4 + 16]
                    inA = res_ap[8 * b4:8 * b4 + 4, (16 * b5 + 6)::32]
                    eng = nc.sync if b4 == 0 else nc.scalar
                    eng.dma_start(rows[:, 0:4].rearrange("u q -> q u"), inA)
                    inB = res_ap[8 * b4 + 3:8 * b4 + 4, (16 * b5)::32]
                    eng.dma_start(rows[:, 4:5].rearrange("u q -> q u"), inB)
                    inC = (res_ap[8 * b4 + 4:8 * b4 + 5]
                           .rearrange("i (u c) -> i u c", c=32)
                           [:, :, 16 * b5 + 1:16 * b5 + 6])
                    eng.dma_start(rows[:, 5:10].unsqueeze(0), inC)

    nc.finalize()
    return nc


def _get_module():
    if "nc" not in _BUILD_CACHE:
        _BUILD_CACHE["nc"] = _build_module()
    return _BUILD_CACHE["nc"]


# ---------------------------------------------------------------- entrypoint
def kernel(inputs, theta):
    inputs = np.asarray(inputs, dtype=np.float32)
    theta = np.asarray(theta, dtype=np.float32)
    assert inputs.shape == (B_TOTAL, N_QUBITS)

    from concourse.bass_utils import run_bass_kernel_spmd

    nc = _get_module()
    wstack = _host_weights(theta)
    in_maps = []
    for c in range(N_CORES):
        shard = np.ascontiguousarray(inputs[B_CORE * c:B_CORE * (c + 1)])
        in_maps.append({"xin": shard, "wstack": wstack})
    res = run_bass_kernel_spmd(nc, in_maps, core_ids=list(range(N_CORES)))
    out = np.concatenate([r["out"] for r in res.results], axis=0)
    return out.astype(np.float32)



# revision 20
# speedup vs baseline: 1.6840x; 1.6840x over previous
"""Trainium2 Bass kernel v2 for nn_EnhancedQuantumLayer (10-qubit, 4-layer
variational circuit, batch 512, Z-expectations output).

Scheme (data parallel over 8 cores, 64 samples/core):
  - Realified complex: re/im is a partition bit c. Layout A partitions
    p = (c, q5, q0..q4), free f = (b4..b1, b0, q6..q9) per half (b5).
    A 32x32 DVE block transpose flips to layout B p = (c, q5, b0, q6..q9),
    f = (b4..b1, q0..q4). One bf16 matmul per (layer, side, half).
  - Feature map: v(t) per (sample, qubit) is an exact 94-term Fourier
    series in t = tanh(x) (odd multiples of 0.25). Built on-device via
    partition_broadcast + per-partition tensor_scalar + range reduce +
    one ACT Sin + one PE contraction; un-transposed by 10 tiny PE
    transposes.
  - Initial state X = (h6 outer l4) built by 2 accumulated matmuls per
    half with the per-sample h6 amplitudes as stationary (k = samples).
  - Measurement: squares + two sign-weight matmuls, all in layout B.

Host precompute is theta-only: 8 realified 128x128 stationaries (bf16).
"""

import math

import ml_dtypes
import numpy as np

N_QUBITS = 10
N_LAYERS = 4
FREQS = (1.0, 2.0, 4.0, 8.0, 16.0)
PI = float(np.pi)
B_TOTAL = 512
B_CORE = 64
N_CORES = 8
WARMUP_MM = 6

CZCNOT = np.array([[1, 0, 0, 0],
                   [0, 1, 0, 0],
                   [0, 0, 0, -1],
                   [0, 0, 1, 0]], dtype=np.complex128)


# ---------------------------------------------------------------- host math
def _rz(phi):
    return np.array([[np.exp(-0.5j * phi), 0], [0, np.exp(0.5j * phi)]])


def _rx(th):
    c, s = np.cos(th / 2), np.sin(th / 2)
    return np.array([[c, -1j * s], [-1j * s, c]])


def _ry(th):
    c, s = np.cos(th / 2), np.sin(th / 2)
    return np.array([[c, -s], [s, c]])


def _kron_list(ms):
    out = ms[0]
    for m in ms[1:]:
        out = np.kron(out, m)
    return out


def _embed_2q(space_qubits, qa, qb, M4):
    n = len(space_qubits)
    dim = 2 ** n
    pa, pb = space_qubits.index(qa), space_qubits.index(qb)
    out = np.zeros((dim, dim), dtype=np.complex128)
    for idx in range(dim):
        bits = [(idx >> (n - 1 - i)) & 1 for i in range(n)]
        col4 = 2 * bits[pa] + bits[pb]
        for row4 in range(4):
            val = M4[row4, col4]
            if val != 0:
                nb = bits.copy()
                nb[pa], nb[pb] = row4 >> 1, row4 & 1
                ridx = sum(bit << (n - 1 - i) for i, bit in enumerate(nb))
                out[ridx, idx] += val
    return out


def _realify(M):
    return np.block([[M.real, -M.imag], [M.imag, M.real]])


def _embed_OB(M_L):
    """layout-B partition op on (q5, b0, q6..q9): identity on b0."""
    M = M_L.reshape(2, 16, 2, 16)
    O = np.zeros((2, 2, 16, 2, 2, 16), np.complex128)
    for b0 in range(2):
        O[:, b0, :, :, b0, :] = M
    return O.reshape(64, 64)


def _host_weights(theta):
    """wstack [128, 8, 128] bf16: per layer [lhsT_A, lhsT_L] with
    lhsT = realify(op).T, partition-major for one contiguous DMA."""
    ang = np.tanh(theta.astype(np.float64)) * PI
    A_space = [5, 0, 1, 2, 3, 4]
    L_space = [5, 6, 7, 8, 9]
    mats = []
    for l in range(N_LAYERS):
        U = []
        for q in range(10):
            a0, a1, a2 = ang[l, q]
            U.append(_rx(a0 * 0.5) @ _rz(a2) @ _ry(a1) @ _rz(a0))
        UA = _kron_list([U[q] for q in A_space])
        E_even_A = (_embed_2q(A_space, 0, 1, CZCNOT)
                    @ _embed_2q(A_space, 2, 3, CZCNOT)
                    @ _embed_2q(A_space, 4, 5, CZCNOT))
        E_odd_A = (_embed_2q(A_space, 1, 2, CZCNOT)
                   @ _embed_2q(A_space, 3, 4, CZCNOT))
        M_A = E_odd_A @ E_even_A @ UA
        UL = _kron_list([np.eye(2)] + [U[q] for q in [6, 7, 8, 9]])
        E_even_L = (_embed_2q(L_space, 6, 7, CZCNOT)
                    @ _embed_2q(L_space, 8, 9, CZCNOT))
        E_odd_L = (_embed_2q(L_space, 5, 6, CZCNOT)
                   @ _embed_2q(L_space, 7, 8, CZCNOT))
        M_L = E_odd_L @ E_even_L @ UL
        mats.append(_realify(M_A).T)
        mats.append(_realify(_embed_OB(M_L)).T)
    stk = np.stack(mats)  # [8, 128, 128]
    return np.ascontiguousarray(
        stk.transpose(1, 0, 2).astype(ml_dtypes.bfloat16))


# ------------------------------------------------------- fourier basis (v)
def _v_of_t(t):
    t = np.atleast_1d(np.asarray(t, np.float64))
    v = np.zeros((t.size, 2), np.complex128)
    v[:, 0] = 1.0
    for f in FREQS:
        phi = f * t
        v = v * np.stack([np.exp(-0.5j * phi), np.exp(0.5j * phi)], -1)
        th = 0.25 * f * t
        c, s = np.cos(th), np.sin(th)
        v = np.stack([c * v[:, 0] - 1j * s * v[:, 1],
                      -1j * s * v[:, 0] + c * v[:, 1]], -1)
    return v


def _fourier_C():
    """C [94, 4]: rows 0-46 sin(0.25 m t), rows 47-93 cos, m = 1,3..93;
    comps (ar, ai, br, bi)."""
    N = 1024
    ts = np.arange(N) * (8 * np.pi / N)
    vv = _v_of_t(ts)
    comps = np.stack([vv[:, 0].real, vv[:, 0].imag,
                      vv[:, 1].real, vv[:, 1].imag], -1)
    F = np.fft.rfft(comps, axis=0)
    msk = np.arange(1, 94, 2)
    a_cos = 2.0 * F[msk].real / N
    b_sin = -2.0 * F[msk].imag / N
    return msk, np.concatenate([b_sin, a_cos], 0).astype(np.float64)


# ------------------------------------------------------------- bass builder
_BUILD_CACHE = {}


def _measurement_weights():
    # W1 [128, 32]: col = b0p*16 + o; o: 0 = one, 1..5 = s5..s9
    W1 = np.zeros((128, 32), np.float32)
    for p in range(128):
        q5 = (p >> 5) & 1
        b0 = (p >> 4) & 1
        j4 = p & 15
        s = [1 - 2 * q5] + [1 - 2 * ((j4 >> (3 - k)) & 1) for k in range(4)]
        W1[p, b0 * 16 + 0] = 1.0
        for k in range(5):
            W1[p, b0 * 16 + 1 + k] = s[k]
    # W2 [32, 8]: m 0 = ones, 1..5 = sg_q0..sg_q4 over j5' = (q0..q4)
    W2 = np.zeros((32, 8), np.float32)
    for j in range(32):
        W2[j, 0] = 1.0
        for q in range(5):
            W2[j, 1 + q] = 1 - 2 * ((j >> (4 - q)) & 1)
    return W1, W2


def _build_module():
    import concourse.bass as bass
    import concourse.mybir as mybir
    from concourse import bacc
    from concourse.tile import TileContext

    f32 = mybir.dt.float32
    f32r = mybir.dt.float32r
    bf16 = mybir.dt.bfloat16
    AF = mybir.ActivationFunctionType
    OP = mybir.AluOpType

    nc = bacc.Bacc("TRN2", target_bir_lowering=False, debug=False)

    xin = nc.dram_tensor("xin", [B_CORE, 10], f32, kind="ExternalInput").ap()
    wstack = nc.dram_tensor("wstack", [128, 8, 128], bf16,
                            kind="ExternalInput").ap()
    out_d = nc.dram_tensor("out", [B_CORE, 10], f32, kind="ExternalOutput").ap()

    # ---- inline constants
    msk, C94 = _fourier_C()
    cpf = np.zeros((128, 16), np.float32)  # f32 consts: omega, bias, C, I4
    cpf[0:47, 0] = 0.25 * msk
    cpf[47:94, 0] = 0.25 * msk
    cpf[47:94, 1] = 0.5 * PI
    cpf[0:94, 2:6] = C94
    cpf[0:4, 6:10] = np.eye(4)
    # scol [64, 2] signs; umask [64, 32]
    cpf[0:64, 10] = -1.0
    cpf[0:64, 11] = 1.0
    cpm = np.zeros((64, 32), np.float32)
    for b in range(64):
        cpm[b, b % 32] = 1.0
    W1, W2 = _measurement_weights()
    cph = np.zeros((128, 32 + 8), np.float32)
    cph[:, 0:32] = W1
    cph[0:32, 32:40] = W2
    cpf_c = nc.inline_tensor(cpf, name="cpf").ap()
    cpm_c = nc.inline_tensor(cpm, name="cpm").ap()
    cph_c = nc.inline_tensor(cph.astype(ml_dtypes.bfloat16), name="cph").ap()

    MAGIC = 1.5 * 2 ** 23
    TWO_PI = 2.0 * PI
    PCLAMP = PI * (1.0 - 1e-6)

    with TileContext(nc) as tc:
        with (
            tc.tile_pool(name="wpool", bufs=1) as wpool,
            tc.tile_pool(name="sm", bufs=2) as sm,
            tc.tile_pool(name="dbl", bufs=2) as db,
            tc.tile_pool(name="xp", bufs=4) as xp,
            tc.tile_pool(name="cv", bufs=4) as cv,
            tc.tile_pool(name="psA", bufs=1, space="PSUM") as psA,
            tc.tile_pool(name="psB", bufs=1, space="PSUM") as psB,
            tc.tile_pool(name="psS", bufs=1, space="PSUM") as psS,
            tc.tile_pool(name="psW", bufs=1, space="PSUM") as psW,
        ):
            # ---- DMAs
            wt = wpool.tile([128, 8 * 128], bf16, tag="w")
            nc.sync.dma_start(
                wt[:].rearrange("p (m c) -> p m c", c=128), wstack)

            ctf = wpool.tile([128, 16], f32, tag="cpf")
            nc.scalar.dma_start(ctf[:], cpf_c)
            ctm = wpool.tile([64, 32], f32, tag="cpm")
            nc.scalar.dma_start(ctm[:], cpm_c)
            cth = wpool.tile([128, 40], bf16, tag="cph")
            nc.scalar.dma_start(cth[:], cph_c)

            omega_c = ctf[:, 0:1]
            bias_c = ctf[:, 1:2]
            cmat = ctf[:, 2:6]
            ident4 = ctf[0:4, 6:10]
            scol = ctf[0:64, 10:12]
            umask = ctm[:]
            w1_t = cth[:, 0:32]
            w2_t = cth[0:32, 32:40]

            # xq: [1, 640] (q-major flatten of xin, transposed DMA)
            xq = sm.tile([1, 640], f32, tag="xq")
            nc.sync.dma_start(
                xq[:].rearrange("p (q b) -> p q b", b=64),
                xin.rearrange("b q -> q b").unsqueeze(0))

            # preload the one ACT table serving Tanh+Sin+Copy+Square
            nc.scalar.add_instruction(
                mybir.InstLoadActFuncSet(
                    name=nc.scalar.bass.get_next_instruction_name(),
                    ins=[], outs=[], act_func_set_id=18))

            def W(m):
                return wt[:, 128 * m:128 * m + 128]

            # ---- PE warmup (keeps HAM un-throttled through the front-end)
            if WARMUP_MM:
                wscr = psW.tile([128, 512], f32, tag="warm")
                for i in range(WARMUP_MM):
                    nc.tensor.matmul(wscr[:], W(0), wt[:, 0:512],
                                     start=True, stop=True,
                                     skip_group_check=True)

            # ---- feature map: broadcast t, args, range-reduce, sin
            x128 = sm.tile([128, 640], f32, tag="x128")
            nc.gpsimd.partition_broadcast(x128[:], xq[:])
            t128 = sm.tile([128, 640], f32, tag="t128")
            nc.scalar.activation(t128[:], x128[:], AF.Tanh)
            args = sm.tile([128, 640], f32, tag="args")
            nc.vector.tensor_scalar(args[:], t128[:], omega_c, bias_c,
                                    OP.mult, OP.add)
            kk = sm.tile([128, 640], f32, tag="kk")
            nc.vector.tensor_scalar(kk[:], args[:], 1.0 / TWO_PI, MAGIC,
                                    OP.mult, OP.add)
            nc.vector.tensor_scalar(kk[:], kk[:], MAGIC, None, OP.subtract)
            nc.vector.scalar_tensor_tensor(args[:], kk[:], -TWO_PI, args[:],
                                           OP.mult, OP.add)
            nc.vector.tensor_scalar(args[:], args[:], PCLAMP, -PCLAMP,
                                    OP.min, OP.max)
            trig = sm.tile([128, 640], f32, tag="trig")
            nc.scalar.activation(trig[:].bitcast(f32r), args[:], AF.Sin)

            # ---- v = C^T trig : 2 matmuls then 10 tiny PE transposes
            vp0 = psS.tile([4, 320], f32, tag="vp0")
            vp1 = psS.tile([4, 320], f32, tag="vp1")
            vps = [vp0, vp1]
            for i in range(2):
                nc.tensor.matmul(vps[i][:], cmat.bitcast(f32r),
                                 trig[:, 320 * i:320 * (i + 1)].bitcast(f32r),
                                 start=True, stop=True)
            vsb = sm.tile([4, 640], f32, tag="vsb")
            for i in range(2):
                nc.scalar.activation(
                    vsb[:, 320 * i:320 * (i + 1)].bitcast(f32r),
                    vps[i][:], AF.Copy)
            vT = psS.tile([64, 40], f32, tag="vT")
            for q in range(10):
                nc.tensor.transpose(
                    vT[:, 4 * q:4 * q + 4].bitcast(f32r),
                    vsb[0:4, 64 * q:64 * q + 64].bitcast(f32r),
                    ident4.bitcast(f32r))
            v_cur = sm.tile([64, 40], f32, tag="vcur")
            nc.vector.tensor_copy(v_cur[:], vT[:])

            # ---- doubling: h6 = v5 x v0..v4 (j6 idx), l4 = v6 x..x v9
            vv = v_cur[:].rearrange("p (q a c) -> p q a c", a=2, c=2)
            vimS = sm.tile([64, 40], f32, tag="vimS")
            vimS_v = vimS[:].rearrange("p (q a c) -> p q a c", a=2, c=2)
            nc.vector.tensor_tensor(
                vimS_v,
                vv[:, :, :, 1:2].broadcast_to((64, 10, 2, 2)),
                scol.unsqueeze(1).unsqueeze(1)
                    .broadcast_to((64, 10, 2, 2))
                    .rearrange("p q a c -> p q a c"),
                OP.mult)

            def vre(q):
                return vv[:, q, :, 0]  # [64, 2]

            def vim_s(q):
                return vimS_v[:, q]  # [64, 2, 2]

            def kstep(eng, Xt, m, Yre, YimS, n, tag, dtype=f32, pool=db):
                """out[p, (j, a, c)] = complex (X kron Y); X [64, m*2]."""
                Xv = Xt[:].rearrange("p (m c) -> p m c", c=2)
                t1 = pool.tile([64, m * n * 2], f32, tag=tag + "1")
                t2 = pool.tile([64, m * n * 2], f32, tag=tag + "2")
                out = pool.tile([64, m * n * 2], dtype, tag=tag)
                t1v = t1[:].rearrange("p (m n c) -> p m n c", m=m, c=2)
                t2v = t2[:].rearrange("p (m n c) -> p m n c", m=m, c=2)
                Xb = Xv.unsqueeze(2).broadcast_to((64, m, n, 2))
                Xsw = (Xv[:, :, ::-1].unsqueeze(2)
                       .broadcast_to((64, m, n, 2)))
                Yreb = (Yre.unsqueeze(1).unsqueeze(3)
                        .broadcast_to((64, m, n, 2)))
                YimSb = YimS.unsqueeze(1).broadcast_to((64, m, n, 2))
                eng.tensor_tensor(t1v, Xb, Yreb, OP.mult)
                eng.tensor_tensor(t2v, Xsw, YimSb, OP.mult)
                eng.tensor_tensor(out[:], t1[:], t2[:], OP.add)
                return out

            def mk_imS(eng, Yt, n, tag, pool=db):
                o = pool.tile([64, n * 2], f32, tag=tag)
                ov = o[:].rearrange("p (n c) -> p n c", c=2)
                Yv = Yt[:].rearrange("p (n c) -> p n c", c=2)
                eng.tensor_tensor(
                    ov, Yv[:, :, 1:2].broadcast_to((64, n, 2)),
                    scol.unsqueeze(1).broadcast_to((64, n, 2)),
                    OP.mult)
                return o

            V = nc.vector
            G = nc.gpsimd
            # v5 as starting tile [64, 4]
            v5t = sm.tile([64, 4], f32, tag="v5t")
            nc.vector.tensor_copy(
                v5t[:].rearrange("p (a c) -> p a c", c=2), vv[:, 5])
            # DVE chain: a = v5 x v0 x v1; c1 = v6 x v7
            a1 = kstep(V, v5t, 2, vre(0), vim_s(0), 2, "a1")
            a2 = kstep(V, a1, 4, vre(1), vim_s(1), 2, "a2")
            v6t = sm.tile([64, 4], f32, tag="v6t")
            nc.vector.tensor_copy(
                v6t[:].rearrange("p (a c) -> p a c", c=2), vv[:, 6])
            c1 = kstep(V, v6t, 2, vre(7), vim_s(7), 2, "c1")
            # GPS chain: b = v2 x v3 x v4 ; c2 = v8 x v9
            v2t = sm.tile([64, 4], f32, tag="v2t")
            nc.gpsimd.tensor_copy(
                v2t[:].rearrange("p (a c) -> p a c", c=2), vv[:, 2])
            b1 = kstep(G, v2t, 2, vre(3), vim_s(3), 2, "b1")
            b2 = kstep(G, b1, 4, vre(4), vim_s(4), 2, "b2")
            b2S = mk_imS(G, b2, 8, "b2S")
            v8t = sm.tile([64, 4], f32, tag="v8t")
            nc.gpsimd.tensor_copy(
                v8t[:].rearrange("p (a c) -> p a c", c=2), vv[:, 8])
            c2 = kstep(G, v8t, 2, vre(9), vim_s(9), 2, "c2")
            c2S = mk_imS(G, c2, 4, "c2S")
            # h6 = a2 x b2 (DVE) written c-major bf16 (stationary layout),
            # l4 = c1 x c2 (GPS)
            b2re = b2[:].rearrange("p (n c) -> p n c", c=2)[:, :, 0]
            b2Sv = b2S[:].rearrange("p (n c) -> p n c", c=2)
            h6t1 = db.tile([64, 128], f32, tag="h61")
            h6t2 = db.tile([64, 128], f32, tag="h62")
            t1v = h6t1[:].rearrange("p (m n c) -> p m n c", m=8, c=2)
            t2v = h6t2[:].rearrange("p (m n c) -> p m n c", m=8, c=2)
            a2v = a2[:].rearrange("p (m c) -> p m c", c=2)
            nc.vector.tensor_tensor(
                t1v, a2v.unsqueeze(2).broadcast_to((64, 8, 8, 2)),
                b2re.unsqueeze(1).unsqueeze(3).broadcast_to((64, 8, 8, 2)),
                OP.mult)
            nc.vector.tensor_tensor(
                t2v, a2v[:, :, ::-1].unsqueeze(2).broadcast_to((64, 8, 8, 2)),
                b2Sv.unsqueeze(1).broadcast_to((64, 8, 8, 2)),
                OP.mult)
            s1bf = sm.tile([64, 128], bf16, tag="s1bf")
            s1w = s1bf[:].rearrange("p (c m n) -> p m n c", c=2, m=8)
            nc.vector.tensor_tensor(
                s1w, t1v, t2v, OP.add)
            c2re = c2[:].rearrange("p (n c) -> p n c", c=2)[:, :, 0]
            l4 = kstep(G, c1, 4, c2re,
                       c2S[:].rearrange("p (n c) -> p n c", c=2), 4, "l4")

            # ---- S2 from S1 (c-major): S2[c] = sign(c) * S1[1-c]
            s1cm = s1bf[:].rearrange("p (c j) -> p c j", c=2)
            s2bf = sm.tile([64, 128], bf16, tag="s2bf")
            nc.vector.tensor_tensor(
                s2bf[:].rearrange("p (c j) -> p c j", c=2),
                s1cm[:, ::-1, :],
                scol.unsqueeze(2).broadcast_to((64, 2, 64)),
                OP.mult)

            l4v = l4[:].rearrange("p (j c) -> p j c", c=2)
            m_ts = []
            for ci in range(2):
                mf = sm.tile([64, 512], f32, tag=f"m{ci}f")
                nc.gpsimd.tensor_tensor(
                    mf[:].rearrange("p (u j) -> p u j", j=16),
                    l4v[:, :, ci].unsqueeze(1).broadcast_to((64, 32, 16)),
                    umask.unsqueeze(2).broadcast_to((64, 32, 16)),
                    OP.mult)
                mbf = sm.tile([64, 512], bf16, tag=f"m{ci}bf")
                nc.scalar.activation(mbf[:], mf[:], AF.Copy)
                m_ts.append(mbf)

            s1view = s1bf[:]

            # ---- X build: per half, 2 accumulated matmuls
            xa = []
            for h in range(2):
                psX = psA.tile([128, 512], f32, tag=f"y{h}")
                nc.tensor.matmul(psX[:],
                                 s1view[32 * h:32 * h + 32],
                                 m_ts[0][32 * h:32 * h + 32, :],
                                 start=True, stop=False)
                nc.tensor.matmul(psX[:],
                                 s2bf[32 * h:32 * h + 32, :],
                                 m_ts[1][32 * h:32 * h + 32, :],
                                 start=False, stop=True)
                xt = xp.tile([128, 512], bf16, tag=f"x{h}")
                if h == 0:
                    nc.scalar.activation(xt[:], psX[:], AF.Copy)
                else:
                    nc.vector.tensor_copy(xt[:], psX[:])
                xa.append(xt)

            # ---- layers
            zb = [None, None]
            for l in range(N_LAYERS):
                for h in range(2):
                    yA = psA.tile([128, 512], f32, tag=f"y{h}")
                    nc.tensor.matmul(yA[:], W(2 * l), xa[h][:],
                                     start=True, stop=True)
                    yc = cv.tile([128, 512], bf16, tag=f"yc{h}")
                    nc.scalar.activation(yc[:], yA[:], AF.Copy)
                    xB = cv.tile([128, 512], bf16, tag=f"xb{h}")
                    nc.vector.transpose(xB[:], yc[:])
                    zB = psB.tile([128, 512], f32, tag=f"z{h}")
                    nc.tensor.matmul(zB[:], W(2 * l + 1), xB[:],
                                     start=True, stop=True)
                    if l < N_LAYERS - 1:
                        zc = cv.tile([128, 512], bf16, tag=f"zc{h}")
                        nc.scalar.activation(zc[:], zB[:], AF.Copy)
                        xt = xp.tile([128, 512], bf16, tag=f"x{h}")
                        nc.vector.transpose(xt[:], zc[:])
                        xa[h] = xt
                    else:
                        zb[h] = zB

            # ---- measurement (layout B)
            outv = out_d.rearrange("(g t) q -> g t q", t=2)
            for h in range(2):
                sq = cv.tile([128, 512], bf16, tag=f"yc{h}")
                nc.scalar.square(sq[:], zb[h][:])
                o1 = psS.tile([32, 512], f32, tag=f"vp{h}")
                nc.tensor.matmul(o1[:], w1_t, sq[:], start=True, stop=True)
                o1c = cv.tile([32, 512], bf16, tag=f"xb{h}")
                nc.scalar.activation(o1c[:], o1[:], AF.Copy)
                o1t = cv.tile([32, 512], bf16, tag=f"zc{h}")
                nc.vector.transpose(o1t[:], o1c[:])
                o2 = psS.tile([8, 512], f32, tag="vT")
                nc.tensor.matmul(o2[:], w2_t, o1t[:], start=True, stop=True)
                res = sm.tile([8, 512], f32, tag=f"res{h}")
                nc.vector.tensor_copy(res[:], o2[:])
                # gather to out[b, q]; b = 32h + 2*bhi + b0
                resv = res[:].rearrange("p (u c) -> p u c", c=32)
                for b0 in range(2):
                    rows = outv[16 * h:16 * h + 16, b0]
                    eng = nc.sync if b0 == 0 else nc.scalar
                    # q5..q9 from row 0 (ones), cols b0*16+1..6
                    eng.dma_start(
                        rows[:, 5:10].unsqueeze(0),
                        resv[0:1, :, 16 * b0 + 1:16 * b0 + 6])
                    # q0..q4 from rows 1..5, col b0*16
                    eng.dma_start(
                        rows[:, 0:5].rearrange("u q -> q u"),
                        resv[1:6, :, 16 * b0])

    nc.finalize()
    return nc


def _get_module():
    if "nc" not in _BUILD_CACHE:
        _BUILD_CACHE["nc"] = _build_module()
    return _BUILD_CACHE["nc"]


# ---------------------------------------------------------------- entrypoint
def kernel(inputs, theta):
    inputs = np.asarray(inputs, dtype=np.float32)
    theta = np.asarray(theta, dtype=np.float32)
    assert inputs.shape == (B_TOTAL, N_QUBITS)

    from concourse.bass_utils import run_bass_kernel_spmd

    nc = _get_module()
    wstack = _host_weights(theta)
    in_maps = []
    for c in range(N_CORES):
        shard = np.ascontiguousarray(inputs[B_CORE * c:B_CORE * (c + 1)])
        in_maps.append({"xin": shard, "wstack": wstack})
    res = run_bass_kernel_spmd(nc, in_maps, core_ids=list(range(N_CORES)))
    out = np.concatenate([r["out"] for r in res.results], axis=0)
    return out.astype(np.float32)
